# revision 29
# baseline (speedup 1.0000x reference)
"""Trainium2 Bass kernel for nn_ConstrainedEnhancementModel.

Contract: kernel(**inputs) takes the FULL unsharded inputs (as produced by
reference.setup_inputs()) and returns the FULL [4096, 2000, 6] float32 output.

Strategy (pure data parallel over 8 NeuronCores, 512 batch rows each):
  - Feature-major MLP chain in fp8 (e4m3) with DoubleRow matmuls: weights are
    scaled x64 into fp8's normal range; b1/b4 are folded into constant-1 K
    rows; h2/h5 are stored pre-scaled x64 so every activation is a single
    scalar-ACT or single vector tensor_scalar op (alternating engines).
  - x arrives host-side pre-transposed twice: compact 5-ktile fp8 (for L1,
    packed with W1 in the wxa blob) and window-blocked bf16 (partition
    32w+r = x col 24*(4*i4+w)+r) in the per-group xbg blob for the
    interpolation matmul.
  - Final layer: out = h5 @ (W6 * c_d * 256 / 64) + x @ (G * 256), evaluated
    per output window (480 cols); fp8 DoubleRow pairs for the W6 part, a K=32
    bf16 matmul on a 32-row PE tile for the G (lin-interp + b6) part -- the
    four window interp matmuls run concurrently on disjoint 32-row strips.
    The psum->sbuf copy applies 1/256 and writes bf16; output DMAs one
    [128, nwin*480] chunk per (group, batch-tile).
  - Output tensor is bf16 (within the rel-err budget); host upcasts to f32.
  - Schedule: 8 warm-up matmuls keep the PE HAM clock at 2.4GHz while the
    wxa blob (big-row DMA) lands; per-group xbg/w6 loads let the final layer
    start as soon as the encoder drains (small group 6 first).
"""

import numpy as np
import ml_dtypes

import bass_rust
import concourse.bass as bass
import concourse.bacc as bacc
import concourse.mybir as mybir
import concourse.tile as tile
from concourse import bass_utils

F32 = mybir.dt.float32
BF16 = mybir.dt.bfloat16
F8 = mybir.dt.float8e4
BF16_NP = ml_dtypes.bfloat16
F8_NP = ml_dtypes.float8_e4m3fn

# Problem config (hardcoded; must match the reference)
LOW_T = 100
HIGH_T = 2000
FEAT = 6
HID = 256
NUM_CLASSES = 10
LBL_DIM = 16
UP = 20
B = 4096
NCORES = 8
BC = B // NCORES          # 512 batch rows per core
NBT = BC // 128           # 4 batch tiles per core
D_IN = LOW_T * FEAT       # 600
D_OUT = HIGH_T * FEAT     # 12000
NW = 25                   # output windows (80 timesteps * 6 feats = 480 cols)
WT = 480
NI4 = 7                   # ceil(25/4) groups of 4 windows
EW = 64.0                 # encoder weight fp8 scale
SC = 256.0                # decoder/W6/G fp8+psum scale
DR = mybir.MatmulPerfMode.DoubleRow
I4_ORDER = [6, 0, 1, 2, 3, 4, 5]   # small group first: w6[g6] lands earliest

# wxa blob (fp8): x8c | W1.  wxb blob (fp8): W2..W5 | l4feat | l4emb.
XO = 0             # x8c: 5 ktiles x 512 (L1 compact: K rows = x col 128k+p)
OW1 = 2560         # W1: 5 ktiles x 512
WXA = 5120
OW2 = 0            # W2: 4 ktiles x 256
OW3 = 1024         # W3: 2 ktiles x 128 (stored x8, not x64)
OW4 = 1280         # W4: 2 ktiles x 256
OW5 = 1792         # W5: 2 ktiles x 512
OL4 = 2816         # L3 output (runtime-written; host sends zeros)
OL4E = 3328        # l4emb ktile (emb rows 0-15, b4 row 16, zeros)
WXB = 3840


def _ap3(t, col_off, stride2, n3):
    """3-dim AP over all 128 partitions of tile t: [128, 2, n3]."""
    a = t[:]
    return bass_rust.AP(
        tensor=a.tensor, offset=a.offset + col_off,
        ap=[[a.ap[0][0], 128], [stride2, 2], [1, n3]],
    )


def _build_nc():
    """Build the single-core Bass program (SPMD: same program on all 8)."""
    nc = bacc.Bacc("TRN2", target_bir_lowering=False, debug=False)

    wxa_d = nc.dram_tensor("wxa", [128, WXA], F8, kind="ExternalInput")
    wxb_d = nc.dram_tensor("wxb", [128, WXB], F8, kind="ExternalInput")
    bias_d = nc.dram_tensor("biasb", [128, 26], F32, kind="ExternalInput")
    xbg_d = nc.dram_tensor("xbg", [128, NI4 * 992], BF16, kind="ExternalInput")
    w6_d = nc.dram_tensor("w6p", [128, NW * 4 * WT], F8, kind="ExternalInput")
    y_d = nc.dram_tensor("y", [BC, D_OUT], BF16, kind="ExternalOutput")

    RELU = mybir.ActivationFunctionType.Relu
    IDENT = mybir.ActivationFunctionType.Identity
    ADD = mybir.AluOpType.add
    MAX = mybir.AluOpType.max
    MULT = mybir.AluOpType.mult

    with tile.TileContext(nc) as tc:
        with (
            tc.tile_pool(name="const", bufs=1) as cp,
            tc.tile_pool(name="outpool", bufs=4) as op,
            tc.tile_pool(name="ppool", bufs=4, space="PSUM") as pm,
        ):
            # ---- persistent SBUF tensors ----
            wxa = cp.tile([128, WXA], F8, tag="wxa", name="wxa")
            wxb = cp.tile([128, WXB], F8, tag="wxb", name="wxb")
            cbias = cp.tile([128, 26], F32, tag="cbias", name="cbias")
            xbg = cp.tile([128, NI4 * 992], BF16, tag="xbg", name="xbg")
            w6all = cp.tile([128, NW * 4 * WT], F8, tag="w6all", name="w6all")
            h1a = cp.tile([128, 2 * BC], F8, tag="h1a", name="h1a")
            h1b = cp.tile([128, 2 * BC], F8, tag="h1b", name="h1b")
            h2 = cp.tile([128, 2 * BC], F8, tag="h2", name="h2")
            h4 = cp.tile([128, 2 * BC], F8, tag="h4", name="h4")
            h5a = cp.tile([128, 2 * BC], F8, tag="h5a", name="h5a")
            h5b = cp.tile([128, 2 * BC], F8, tag="h5b", name="h5b")
            dmy = cp.tile([128, 512], F8, tag="dmy", name="dmy")

            # warm-up operand: memset on the (otherwise idle) vector engine
            nc.vector.memset(dmy[:], 0.0)

            # ---- loads, ordered by first use ----
            nc.sync.dma_start(wxa[:], wxa_d[:])
            nc.sync.dma_start(cbias[:], bias_d[:])
            nc.sync.dma_start(wxb[:], wxb_d[:])
            for g in I4_ORDER:
                nwin = 4 if g < 6 else 1
                nc.sync.dma_start(
                    xbg[:, 992 * g:992 * g + 992], xbg_d[:, 992 * g:992 * g + 992]
                )
                o = g * 4 * WT * 4
                nc.sync.dma_start(
                    w6all[:, o:o + nwin * 4 * WT], w6_d[:, o:o + nwin * 4 * WT]
                )

            # bias column views (f32): col 0 = 0.0, 17-18 = 64*b2,
            # 19 = b3 raw, 22-25 = 64*b5
            zb = cbias[:, 0:1]
            vb2 = [cbias[:, 17 + m:18 + m] for m in range(2)]
            vb3 = cbias[:, 19:20]
            vb5 = [cbias[:, 22 + m:23 + m] for m in range(4)]

            def act_s0(dst, ps):
                # relu(ps)/64 on the scalar engine (bias folded into matmul)
                nc.scalar.activation(dst, ps, RELU, bias=zb, scale=1.0 / EW)

            def act_v0(dst, ps):
                # relu(ps)/64 on the vector engine
                nc.vector.tensor_scalar(dst, ps, 0.0, 1.0 / EW, MAX, MULT)

            def act_sb(dst, ps, vb):
                # relu(ps + 64b): output stays pre-scaled x64
                nc.scalar.activation(dst, ps, RELU, bias=vb, scale=1.0)

            def act_vb(dst, ps, vb):
                nc.vector.tensor_scalar(dst, ps, vb, 0.0, ADD, MAX)

            # ---- PE warm-up: sustained activity so HAM unthrottles to 2.4GHz
            # before L1 starts and while the wxa DMA lands (12 x N=512 cold ~= 5.1us)
            for _ in range(12):
                psd = pm.tile([128, 1024], F32, tag="ps", name="ps")
                nc.tensor.matmul(psd[:, 0:512], dmy[:, 0:128], dmy[:],
                                 start=True, stop=True)

            # ---- encoder MLP (feature-major, fp8 DoubleRow) ----
            # L1: [600->512] compact x, 5 ktiles = 2 DR pairs + 1 plain
            h1t = [h1a, h1a, h1b, h1b]
            for m in range(4):
                ps = pm.tile([128, 1024], F32, tag="ps", name="ps")
                for p in range(2):
                    nc.tensor.matmul(
                        ps[:, 0:BC],
                        _ap3(wxa, OW1 + 2 * p * 512 + m * 128, 512, 128),
                        _ap3(wxa, XO + 2 * p * 512, 512, 512),
                        start=(p == 0), stop=False, perf_mode=DR,
                    )
                nc.tensor.matmul(
                    ps[:, 0:BC],
                    wxa[:, OW1 + 4 * 512 + m * 128:OW1 + 4 * 512 + (m + 1) * 128],
                    wxa[:, XO + 4 * 512:XO + 5 * 512], start=False, stop=True,
                )
                dst = h1t[m][:, (m % 2) * BC:(m % 2 + 1) * BC]
                if m % 2 == 0:
                    act_s0(dst, ps[:, 0:BC])
                else:
                    act_v0(dst, ps[:, 0:BC])
            # L2: [512->256], 4 ktiles = 2 DR pairs; h2 stored x64
            for m in range(2):
                ps = pm.tile([128, 1024], F32, tag="ps", name="ps")
                for p, hsrc in enumerate((h1a, h1b)):
                    nc.tensor.matmul(
                        ps[:, 0:BC],
                        _ap3(wxb, OW2 + 2 * p * 256 + m * 128, 256, 128),
                        _ap3(hsrc, 0, BC, 512),
                        start=(p == 0), stop=(p == 1), perf_mode=DR,
                    )
                dst = h2[:, m * BC:(m + 1) * BC]
                if m == 0:
                    act_sb(dst, ps[:, 0:BC], vb2[m])
                else:
                    act_vb(dst, ps[:, 0:BC], vb2[m])
            # L3: [256->128] no relu -> wxb l4feat ktile (single vector op)
            # psum = 8*64*(h2@W3); feat = psum/512 + b3
            ps = pm.tile([128, 1024], F32, tag="ps", name="ps")
            nc.tensor.matmul(
                ps[:, 0:BC], _ap3(wxb, OW3, 128, 128), _ap3(h2, 0, BC, 512),
                start=True, stop=True, perf_mode=DR,
            )
            nc.vector.tensor_scalar(wxb[:, OL4:OL4 + BC], ps[:, 0:BC],
                                    1.0 / (8.0 * EW), vb3, MULT, ADD)
            # L4: [144->256] (feat ktile + padded label ktile, b4 folded)
            for m in range(2):
                ps = pm.tile([128, 1024], F32, tag="ps", name="ps")
                nc.tensor.matmul(
                    ps[:, 0:BC], _ap3(wxb, OW4 + m * 128, 256, 128),
                    _ap3(wxb, OL4, 512, 512),
                    start=True, stop=True, perf_mode=DR,
                )
                dst = h4[:, m * BC:(m + 1) * BC]
                if m == 0:
                    act_s0(dst, ps[:, 0:BC])
                else:
                    act_v0(dst, ps[:, 0:BC])
            # L5: [256->512]; h5 stored x64 in split tiles
            h5t = [h5a, h5a, h5b, h5b]
            for m in range(4):
                ps = pm.tile([128, 1024], F32, tag="ps", name="ps")
                nc.tensor.matmul(
                    ps[:, 0:BC], _ap3(wxb, OW5 + m * 128, 512, 128),
                    _ap3(h4, 0, BC, 512),
                    start=True, stop=True, perf_mode=DR,
                )
                dst = h5t[m][:, (m % 2) * BC:(m % 2 + 1) * BC]
                if m % 2 == 0:
                    act_sb(dst, ps[:, 0:BC], vb5[m])
                else:
                    act_vb(dst, ps[:, 0:BC], vb5[m])

            # ---- final layer + fused constraint epilogue ----
            # Per (group, batch-tile): two 2-bank psum tiles hold the 4
            # windows (w0,w1 in psA banks, w2,w3 in psB banks); the psum->ob
            # copy is ONE strided-AP op per psum tile (scalar: psA, vector:
            # psB) so each engine runs ~55% busy and never backpressures PE.
            for i4 in I4_ORDER:
                nwin = 4 if i4 < 6 else 1
                for bt in range(NBT):
                    psA = pm.tile([128, 1024], F32, tag="ps", name="ps")
                    psB = psA if nwin == 1 else \
                        pm.tile([128, 1024], F32, tag="ps", name="ps")
                    pst = [psA, psA, psB, psB]
                    pss = [pst[w][:, (w % 2) * 512:(w % 2) * 512 + WT]
                           for w in range(nwin)]
                    for k2, h5s in enumerate((h5a, h5b)):
                        for w in range(nwin):
                            nc.tensor.matmul(
                                pss[w],
                                _ap3(h5s, bt * 128, BC, 128),
                                _ap3(w6all, (i4 * 4 + w) * 4 * WT + k2 * 2 * WT, WT, WT),
                                start=(k2 == 0), stop=False, perf_mode=DR,
                            )
                    for w in range(nwin):
                        p0 = 32 * w
                        nc.tensor.matmul(
                            pss[w],
                            xbg[p0:p0 + 32, 992 * i4 + bt * 128:992 * i4 + bt * 128 + 128],
                            xbg[p0:p0 + 32, 992 * i4 + 512:992 * i4 + 992],
                            start=False, stop=True, tile_position=(p0, 0),
                        )
                    ob = op.tile([128, nwin * WT], BF16, tag=f"ob{nwin}", name=f"ob{nwin}")
                    if nwin == 1:
                        nc.scalar.mul(ob[:], pss[0], 1.0 / SC)
                    else:
                        nc.scalar.mul(_ap3(ob, 0, WT, WT),
                                      _ap3(psA, 0, 512, WT), 1.0 / SC)
                        nc.vector.tensor_scalar_mul(_ap3(ob, 2 * WT, WT, WT),
                                                    _ap3(psB, 0, 512, WT), 1.0 / SC)
                    nc.sync.dma_start(
                        y_d[bt * 128:(bt + 1) * 128,
                            i4 * 4 * WT:i4 * 4 * WT + nwin * WT],
                        ob[:],
                    )

    nc.compile()
    return nc


def _host_prep(inputs):
    """Build per-core in_maps from the full inputs."""
    x_full = np.asarray(inputs["low_res_data"], np.float32).reshape(B, D_IN)
    labels = np.asarray(inputs["labels"]).astype(np.int64)
    emb = np.asarray(inputs["emb"], np.float32)
    W6 = np.asarray(inputs["W6"], np.float32)
    b6 = np.asarray(inputs["b6"], np.float32)

    # per-timestep blend coefficients (match the reference formulas)
    t = np.arange(HIGH_T)
    seg = np.clip(t // UP, 0, LOW_T - 2)
    alpha = ((t - seg * UP) / UP).astype(np.float64)
    is_anchor = (t % UP) == 0
    interior = t < (LOW_T - 1) * UP
    blendf = np.where(is_anchor, 1.0, np.where(interior, 0.8, 0.0))
    c_d = np.where(is_anchor, 0.0, np.where(interior, 0.2, 1.0))
    c_start = blendf * (1.0 - alpha) * SC
    c_end = blendf * alpha * SC

    # G matrix, window-blocked: [128, NI4*480]; window i at partition
    # offset 32*(i%4), col block i//4.  Rows r=0..29 <-> x col 24*i + r,
    # row 30 = bias row (pairs with the 1.0 row of the x layout).
    gmat = np.zeros((128, NI4 * WT), np.float64)
    for tt in range(HIGH_T):
        i, dt = divmod(tt, 80)
        i4, wpos = divmod(i, 4)
        p0 = 32 * wpos
        sl = seg[tt] - 4 * i
        for f in range(FEAT):
            col = i4 * WT + FEAT * dt + f
            gmat[p0 + FEAT * sl + f, col] += c_start[tt]
            gmat[p0 + FEAT * (sl + 1) + f, col] += c_end[tt]
            gmat[p0 + 30, col] = c_d[tt] * SC * np.float64(b6[FEAT * tt + f])
    gmat = gmat.astype(np.float32).astype(BF16_NP)

    # W6 blob: [128, 100*480] fp8; window i block at col (i4*4+w)*1920,
    # sub-blocks [k2][ko] of 480 cols = W6 ktile (2*k2+ko) for that window.
    # h5 arrives pre-scaled x64, so the fp8 weight carries c_d*SC/64.
    c_d_full = np.repeat(c_d, FEAT).astype(np.float32)
    w6s = (W6 * (c_d_full * SC / EW)[None, :]).astype(np.float32)
    w6r = w6s.reshape(4, 128, NW, WT)
    w6blob = np.zeros((128, NW * 4 * WT), np.float32)
    for i in range(NW):
        i4, w = divmod(i, 4)
        for kt in range(4):
            o = (i4 * 4 + w) * 4 * WT + kt * WT
            w6blob[:, o:o + WT] = w6r[kt, :, i, :]
    w6blob = w6blob.astype(F8_NP)

    # wxa shared part: W1 (x64) + folded b1 row
    wxas = np.zeros((128, WXA), np.float32)
    W1 = np.asarray(inputs["W1"], np.float32)
    for kt in range(5):
        nr = min(128, D_IN - 128 * kt)
        wxas[:nr, OW1 + kt * 512:OW1 + (kt + 1) * 512] = \
            W1[128 * kt:128 * kt + nr, :] * EW
    wxas[88, OW1 + 4 * 512:OW1 + 5 * 512] = \
        np.asarray(inputs["b1"], np.float32) * EW
    wxas = wxas.astype(F8_NP)

    # wxb shared part: W2 (x64), W3 (x8), W4 (x64, b4 folded), W5 (x64)
    wxbs = np.zeros((128, WXB), np.float32)
    W2 = np.asarray(inputs["W2"], np.float32) * EW
    for kt in range(4):
        wxbs[:, OW2 + kt * 256:OW2 + (kt + 1) * 256] = W2[kt * 128:(kt + 1) * 128]
    W3 = np.asarray(inputs["W3"], np.float32) * 8.0
    for kt in range(2):
        wxbs[:, OW3 + kt * 128:OW3 + (kt + 1) * 128] = W3[kt * 128:(kt + 1) * 128]
    W4 = np.asarray(inputs["W4"], np.float32) * EW
    wxbs[:, OW4:OW4 + 256] = W4[:128]
    wxbs[0:16, OW4 + 256:OW4 + 512] = W4[128:144]
    wxbs[16, OW4 + 256:OW4 + 512] = np.asarray(inputs["b4"], np.float32) * EW
    W5 = np.asarray(inputs["W5"], np.float32) * EW
    for kt in range(2):
        wxbs[:, OW5 + kt * 512:OW5 + (kt + 1) * 512] = W5[kt * 128:(kt + 1) * 128]
    wxbs = wxbs.astype(F8_NP)

    # bias blob [128, 26] f32: col 0 zero, 17-18 = 64*b2, 19 = b3, 22-25 = 64*b5
    biasb = np.zeros((128, 26), np.float32)
    biasb[:, 17:19] = np.asarray(inputs["b2"], np.float32).reshape(2, 128).T * EW
    biasb[:, 19] = np.asarray(inputs["b3"], np.float32)
    biasb[:, 22:26] = np.asarray(inputs["b5"], np.float32).reshape(4, 128).T * EW

    in_maps = []
    for c in range(NCORES):
        sl = slice(c * BC, (c + 1) * BC)
        xc = x_full[sl]  # [BC, 600]
        xw = np.zeros((128, NI4 * 512), np.float32)
        for i in range(NW):
            i4, wpos = divmod(i, 4)
            p0 = 32 * wpos
            ncols = min(30, D_IN - 24 * i)
            xw[p0:p0 + ncols, i4 * 512:i4 * 512 + BC] = xc[:, 24 * i:24 * i + ncols].T
            xw[p0 + 30, i4 * 512:i4 * 512 + BC] = 1.0
        # xbg blob: per-group blocks [x_g (512) | G_g (480)] for split loads
        xbg = np.zeros((128, NI4 * 992), BF16_NP)
        for g in range(NI4):
            xbg[:, 992 * g:992 * g + 512] = xw[:, 512 * g:512 * (g + 1)].astype(BF16_NP)
            xbg[:, 992 * g + 512:992 * (g + 1)] = gmat[:, WT * g:WT * (g + 1)]
        # wxa: compact x for L1 (ktile k = x cols 128k..128k+127) + W1
        wxa = wxas.copy()
        x8c = np.zeros((128, 5 * 512), np.float32)
        for kt in range(5):
            nr = min(128, D_IN - 128 * kt)
            x8c[:nr, kt * 512:kt * 512 + BC] = xc[:, 128 * kt:128 * kt + nr].T
        x8c[88, 4 * 512:5 * 512] = 1.0  # b1 row
        wxa[:, XO:XO + 2560] = x8c.astype(F8_NP)
        # wxb: shared weights + per-core l4emb ktile
        wxb = wxbs.copy()
        l4emb = np.zeros((128, BC), np.float32)
        l4emb[0:LBL_DIM] = emb[labels[sl]].T
        l4emb[16] = 1.0  # b4 row
        wxb[:, OL4E:OL4E + BC] = l4emb.astype(F8_NP)
        m = {"biasb": biasb, "w6p": w6blob, "wxa": wxa, "wxb": wxb,
             "xbg": xbg}
        in_maps.append(m)
    return in_maps


_NC_CACHE = None


def kernel(**inputs) -> np.ndarray:
    global _NC_CACHE
    if _NC_CACHE is None:
        _NC_CACHE = _build_nc()
    nc = _NC_CACHE
    in_maps = _host_prep(inputs)
    res = bass_utils.run_bass_kernel_spmd(nc, in_maps, core_ids=list(range(NCORES)))
    out = np.concatenate([res.results[c]["y"] for c in range(NCORES)], axis=0)
    return out.astype(np.float32).reshape(B, HIGH_T, FEAT)


# revision 31
# speedup vs baseline: 1.0714x; 1.0714x over previous
"""Trainium2 Bass kernel for nn_ConstrainedEnhancementModel.

Contract: kernel(**inputs) takes the FULL unsharded inputs (as produced by
reference.setup_inputs()) and returns the FULL [4096, 2000, 6] float32 output.

Strategy (pure data parallel over 8 NeuronCores, 512 batch rows each):
  - Feature-major MLP chain in fp8 (e4m3) with DoubleRow matmuls: weights are
    scaled x64 into fp8's normal range; b1/b4 are folded into constant-1 K
    rows; h2/h5 are stored pre-scaled x64 so every activation is a single
    scalar-ACT or single vector tensor_scalar op (alternating engines).
  - x arrives host-side pre-transposed twice: compact 5-ktile fp8 (for L1,
    packed with W1 in the wxa blob) and window-blocked bf16 (partition
    32w+r = x col 24*(4*i4+w)+r) in the per-group xbg blob for the
    interpolation matmul.
  - Final layer: out = h5 @ (W6 * c_d * 256 / 64) + x @ (G * 256), evaluated
    per output window (480 cols); fp8 DoubleRow pairs for the W6 part, a K=32
    bf16 matmul on a 32-row PE tile for the G (lin-interp + b6) part -- the
    four window interp matmuls run concurrently on disjoint 32-row strips.
    The psum->sbuf copy applies 1/256 and writes bf16; output DMAs one
    [128, nwin*480] chunk per (group, batch-tile).
  - Output tensor is bf16 (within the rel-err budget); host upcasts to f32.
  - Schedule: 8 warm-up matmuls keep the PE HAM clock at 2.4GHz while the
    wxa blob (big-row DMA) lands; per-group xbg/w6 loads let the final layer
    start as soon as the encoder drains (small group 6 first).
"""

import numpy as np
import ml_dtypes

import bass_rust
import concourse.bass as bass
import concourse.bacc as bacc
import concourse.mybir as mybir
import concourse.tile as tile
from concourse import bass_utils

F32 = mybir.dt.float32
BF16 = mybir.dt.bfloat16
F8 = mybir.dt.float8e4
BF16_NP = ml_dtypes.bfloat16
F8_NP = ml_dtypes.float8_e4m3fn

# Problem config (hardcoded; must match the reference)
LOW_T = 100
HIGH_T = 2000
FEAT = 6
HID = 256
NUM_CLASSES = 10
LBL_DIM = 16
UP = 20
B = 4096
NCORES = 8
BC = B // NCORES          # 512 batch rows per core
NBT = BC // 128           # 4 batch tiles per core
D_IN = LOW_T * FEAT       # 600
D_OUT = HIGH_T * FEAT     # 12000
NW = 25                   # output windows (80 timesteps * 6 feats = 480 cols)
WT = 480
NI4 = 7                   # ceil(25/4) groups of 4 windows
EW = 64.0                 # encoder weight fp8 scale
SC = 256.0                # decoder/W6/G fp8+psum scale
DR = mybir.MatmulPerfMode.DoubleRow
I4_ORDER = [6, 0, 1, 2, 3, 4, 5]   # small group first: w6[g6] lands earliest

# wxa blob (fp8): x8c | W1.  wxb blob (fp8): W2..W5 | l4feat | l4emb.
XO = 0             # x8c: 5 ktiles x 512 (L1 compact: K rows = x col 128k+p)
OW1 = 2560         # W1: 5 ktiles x 512
WXA = 5120
OW2 = 0            # W2: 4 ktiles x 256
OW3 = 1024         # W3: 2 ktiles x 128 (stored x8, not x64)
OW4 = 1280         # W4: 2 ktiles x 256
OW5 = 1792         # W5: 2 ktiles x 512
OL4 = 2816         # L3 output (runtime-written; host sends zeros)
OL4E = 3328        # l4emb ktile (emb rows 0-15, b4 row 16, zeros)
WXB = 3840


def _ap3(t, col_off, stride2, n3):
    """3-dim AP over all 128 partitions of tile t: [128, 2, n3]."""
    a = t[:]
    return bass_rust.AP(
        tensor=a.tensor, offset=a.offset + col_off,
        ap=[[a.ap[0][0], 128], [stride2, 2], [1, n3]],
    )


def _build_nc():
    """Build the single-core Bass program (SPMD: same program on all 8)."""
    nc = bacc.Bacc("TRN2", target_bir_lowering=False, debug=False)

    wxa_d = nc.dram_tensor("wxa", [128, WXA], F8, kind="ExternalInput")
    wxb_d = nc.dram_tensor("wxb", [128, WXB], F8, kind="ExternalInput")
    bias_d = nc.dram_tensor("biasb", [128, 26], F32, kind="ExternalInput")
    xbg_d = nc.dram_tensor("xbg", [128, NI4 * 992], BF16, kind="ExternalInput")
    w6_d = nc.dram_tensor("w6p", [128, NW * 4 * WT], F8, kind="ExternalInput")
    y_d = nc.dram_tensor("y", [BC, D_OUT], BF16, kind="ExternalOutput")

    RELU = mybir.ActivationFunctionType.Relu
    IDENT = mybir.ActivationFunctionType.Identity
    ADD = mybir.AluOpType.add
    MAX = mybir.AluOpType.max
    MULT = mybir.AluOpType.mult

    with tile.TileContext(nc) as tc:
        with (
            tc.tile_pool(name="const", bufs=1) as cp,
            tc.tile_pool(name="outpool", bufs=4) as op,
            tc.tile_pool(name="ppool", bufs=4, space="PSUM") as pm,
        ):
            # ---- persistent SBUF tensors ----
            wxa = cp.tile([128, WXA], F8, tag="wxa", name="wxa")
            wxb = cp.tile([128, WXB], F8, tag="wxb", name="wxb")
            cbias = cp.tile([128, 26], F32, tag="cbias", name="cbias")
            xbg = cp.tile([128, NI4 * 992], BF16, tag="xbg", name="xbg")
            w6all = cp.tile([128, NW * 4 * WT], F8, tag="w6all", name="w6all")
            h1a = cp.tile([128, 2 * BC], F8, tag="h1a", name="h1a")
            h1b = cp.tile([128, 2 * BC], F8, tag="h1b", name="h1b")
            h2 = cp.tile([128, 2 * BC], F8, tag="h2", name="h2")
            h4 = cp.tile([128, 2 * BC], F8, tag="h4", name="h4")
            h5a = cp.tile([128, 2 * BC], F8, tag="h5a", name="h5a")
            h5b = cp.tile([128, 2 * BC], F8, tag="h5b", name="h5b")
            dmy = cp.tile([128, 512], F8, tag="dmy", name="dmy")

            # warm-up operand: memset on the (otherwise idle) vector engine
            nc.vector.memset(dmy[:], 0.0)

            # ---- loads, ordered by first use ----
            nc.sync.dma_start(wxa[:], wxa_d[:])
            nc.sync.dma_start(cbias[:], bias_d[:])
            nc.sync.dma_start(wxb[:], wxb_d[:])
            for g in I4_ORDER:
                nwin = 4 if g < 6 else 1
                nc.sync.dma_start(
                    xbg[:, 992 * g:992 * g + 992], xbg_d[:, 992 * g:992 * g + 992]
                )
                o = g * 4 * WT * 4
                nc.sync.dma_start(
                    w6all[:, o:o + nwin * 4 * WT], w6_d[:, o:o + nwin * 4 * WT]
                )

            # bias column views (f32): col 0 = 0.0, 17-18 = 64*b2,
            # 19 = b3 raw, 22-25 = 64*b5
            zb = cbias[:, 0:1]
            vb2 = [cbias[:, 17 + m:18 + m] for m in range(2)]
            vb3 = cbias[:, 19:20]
            vb5 = [cbias[:, 22 + m:23 + m] for m in range(4)]

            def act_s0(dst, ps):
                # relu(ps)/64 on the scalar engine (bias folded into matmul)
                nc.scalar.activation(dst, ps, RELU, bias=zb, scale=1.0 / EW)

            def act_v0(dst, ps):
                # relu(ps)/64 on the vector engine
                nc.vector.tensor_scalar(dst, ps, 0.0, 1.0 / EW, MAX, MULT)

            def act_sb(dst, ps, vb):
                # relu(ps + 64b): output stays pre-scaled x64
                nc.scalar.activation(dst, ps, RELU, bias=vb, scale=1.0)

            def act_vb(dst, ps, vb):
                nc.vector.tensor_scalar(dst, ps, vb, 0.0, ADD, MAX)

            def warm(n):
                # dummy matmuls: fill PE idle windows (act/DMA waits) so the
                # HAM activity monitor never re-throttles the clock to 1.2GHz
                for _ in range(n):
                    psd = pm.tile([128, 1024], F32, tag="ps", name="ps")
                    nc.tensor.matmul(psd[:, 0:512], dmy[:, 0:128], dmy[:],
                                     start=True, stop=True)

            # ---- PE warm-up: sustained activity so HAM unthrottles to 2.4GHz
            # before L1 starts and while the wxa DMA lands (12 x N=512 cold ~= 5.1us)
            warm(12)

            # ---- encoder MLP (feature-major, fp8 DoubleRow) ----
            # L1: [600->512] compact x, 5 ktiles = 2 DR pairs + 1 plain
            h1t = [h1a, h1a, h1b, h1b]
            for m in range(4):
                ps = pm.tile([128, 1024], F32, tag="ps", name="ps")
                for p in range(2):
                    nc.tensor.matmul(
                        ps[:, 0:BC],
                        _ap3(wxa, OW1 + 2 * p * 512 + m * 128, 512, 128),
                        _ap3(wxa, XO + 2 * p * 512, 512, 512),
                        start=(p == 0), stop=False, perf_mode=DR,
                    )
                nc.tensor.matmul(
                    ps[:, 0:BC],
                    wxa[:, OW1 + 4 * 512 + m * 128:OW1 + 4 * 512 + (m + 1) * 128],
                    wxa[:, XO + 4 * 512:XO + 5 * 512], start=False, stop=True,
                )
                dst = h1t[m][:, (m % 2) * BC:(m % 2 + 1) * BC]
                if m % 2 == 0:
                    act_s0(dst, ps[:, 0:BC])
                else:
                    act_v0(dst, ps[:, 0:BC])
            warm(3)
            # L2: [512->256], 4 ktiles = 2 DR pairs; h2 stored x64
            for m in range(2):
                ps = pm.tile([128, 1024], F32, tag="ps", name="ps")
                for p, hsrc in enumerate((h1a, h1b)):
                    nc.tensor.matmul(
                        ps[:, 0:BC],
                        _ap3(wxb, OW2 + 2 * p * 256 + m * 128, 256, 128),
                        _ap3(hsrc, 0, BC, 512),
                        start=(p == 0), stop=(p == 1), perf_mode=DR,
                    )
                dst = h2[:, m * BC:(m + 1) * BC]
                if m == 0:
                    act_sb(dst, ps[:, 0:BC], vb2[m])
                else:
                    act_vb(dst, ps[:, 0:BC], vb2[m])
            warm(3)
            # L3: [256->128] no relu -> wxb l4feat ktile (single vector op)
            # psum = 8*64*(h2@W3); feat = psum/512 + b3
            ps = pm.tile([128, 1024], F32, tag="ps", name="ps")
            nc.tensor.matmul(
                ps[:, 0:BC], _ap3(wxb, OW3, 128, 128), _ap3(h2, 0, BC, 512),
                start=True, stop=True, perf_mode=DR,
            )
            nc.vector.tensor_scalar(wxb[:, OL4:OL4 + BC], ps[:, 0:BC],
                                    1.0 / (8.0 * EW), vb3, MULT, ADD)
            warm(3)
            # L4: [144->256] (feat ktile + padded label ktile, b4 folded)
            for m in range(2):
                ps = pm.tile([128, 1024], F32, tag="ps", name="ps")
                nc.tensor.matmul(
                    ps[:, 0:BC], _ap3(wxb, OW4 + m * 128, 256, 128),
                    _ap3(wxb, OL4, 512, 512),
                    start=True, stop=True, perf_mode=DR,
                )
                dst = h4[:, m * BC:(m + 1) * BC]
                if m == 0:
                    act_s0(dst, ps[:, 0:BC])
                else:
                    act_v0(dst, ps[:, 0:BC])
            warm(3)
            # L5: [256->512]; h5 stored x64 in split tiles
            h5t = [h5a, h5a, h5b, h5b]
            for m in range(4):
                ps = pm.tile([128, 1024], F32, tag="ps", name="ps")
                nc.tensor.matmul(
                    ps[:, 0:BC], _ap3(wxb, OW5 + m * 128, 512, 128),
                    _ap3(h4, 0, BC, 512),
                    start=True, stop=True, perf_mode=DR,
                )
                dst = h5t[m][:, (m % 2) * BC:(m % 2 + 1) * BC]
                if m % 2 == 0:
                    act_sb(dst, ps[:, 0:BC], vb5[m])
                else:
                    act_vb(dst, ps[:, 0:BC], vb5[m])

            warm(3)

            # ---- final layer + fused constraint epilogue ----
            # Per (group, batch-tile): two 2-bank psum tiles hold the 4
            # windows (w0,w1 in psA banks, w2,w3 in psB banks); the psum->ob
            # copy is ONE strided-AP op per psum tile (scalar: psA, vector:
            # psB) so each engine runs ~55% busy and never backpressures PE.
            for i4 in I4_ORDER:
                nwin = 4 if i4 < 6 else 1
                for bt in range(NBT):
                    psA = pm.tile([128, 1024], F32, tag="ps", name="ps")
                    psB = psA if nwin == 1 else \
                        pm.tile([128, 1024], F32, tag="ps", name="ps")
                    pst = [psA, psA, psB, psB]
                    pss = [pst[w][:, (w % 2) * 512:(w % 2) * 512 + WT]
                           for w in range(nwin)]
                    for k2, h5s in enumerate((h5a, h5b)):
                        for w in range(nwin):
                            nc.tensor.matmul(
                                pss[w],
                                _ap3(h5s, bt * 128, BC, 128),
                                _ap3(w6all, (i4 * 4 + w) * 4 * WT + k2 * 2 * WT, WT, WT),
                                start=(k2 == 0), stop=False, perf_mode=DR,
                            )
                    for w in range(nwin):
                        p0 = 32 * w
                        nc.tensor.matmul(
                            pss[w],
                            xbg[p0:p0 + 32, 992 * i4 + bt * 128:992 * i4 + bt * 128 + 128],
                            xbg[p0:p0 + 32, 992 * i4 + 512:992 * i4 + 992],
                            start=False, stop=True, tile_position=(p0, 0),
                        )
                    ob = op.tile([128, nwin * WT], BF16, tag=f"ob{nwin}", name=f"ob{nwin}")
                    if nwin == 1:
                        nc.scalar.mul(ob[:], pss[0], 1.0 / SC)
                    else:
                        nc.scalar.mul(_ap3(ob, 0, WT, WT),
                                      _ap3(psA, 0, 512, WT), 1.0 / SC)
                        nc.vector.tensor_scalar_mul(_ap3(ob, 2 * WT, WT, WT),
                                                    _ap3(psB, 0, 512, WT), 1.0 / SC)
                    nc.sync.dma_start(
                        y_d[bt * 128:(bt + 1) * 128,
                            i4 * 4 * WT:i4 * 4 * WT + nwin * WT],
                        ob[:],
                    )

    nc.compile()
    return nc


def _host_prep(inputs):
    """Build per-core in_maps from the full inputs."""
    x_full = np.asarray(inputs["low_res_data"], np.float32).reshape(B, D_IN)
    labels = np.asarray(inputs["labels"]).astype(np.int64)
    emb = np.asarray(inputs["emb"], np.float32)
    W6 = np.asarray(inputs["W6"], np.float32)
    b6 = np.asarray(inputs["b6"], np.float32)

    # per-timestep blend coefficients (match the reference formulas)
    t = np.arange(HIGH_T)
    seg = np.clip(t // UP, 0, LOW_T - 2)
    alpha = ((t - seg * UP) / UP).astype(np.float64)
    is_anchor = (t % UP) == 0
    interior = t < (LOW_T - 1) * UP
    blendf = np.where(is_anchor, 1.0, np.where(interior, 0.8, 0.0))
    c_d = np.where(is_anchor, 0.0, np.where(interior, 0.2, 1.0))
    c_start = blendf * (1.0 - alpha) * SC
    c_end = blendf * alpha * SC

    # G matrix, window-blocked: [128, NI4*480]; window i at partition
    # offset 32*(i%4), col block i//4.  Rows r=0..29 <-> x col 24*i + r,
    # row 30 = bias row (pairs with the 1.0 row of the x layout).
    gmat = np.zeros((128, NI4 * WT), np.float64)
    for tt in range(HIGH_T):
        i, dt = divmod(tt, 80)
        i4, wpos = divmod(i, 4)
        p0 = 32 * wpos
        sl = seg[tt] - 4 * i
        for f in range(FEAT):
            col = i4 * WT + FEAT * dt + f
            gmat[p0 + FEAT * sl + f, col] += c_start[tt]
            gmat[p0 + FEAT * (sl + 1) + f, col] += c_end[tt]
            gmat[p0 + 30, col] = c_d[tt] * SC * np.float64(b6[FEAT * tt + f])
    gmat = gmat.astype(np.float32).astype(BF16_NP)

    # W6 blob: [128, 100*480] fp8; window i block at col (i4*4+w)*1920,
    # sub-blocks [k2][ko] of 480 cols = W6 ktile (2*k2+ko) for that window.
    # h5 arrives pre-scaled x64, so the fp8 weight carries c_d*SC/64.
    c_d_full = np.repeat(c_d, FEAT).astype(np.float32)
    w6s = (W6 * (c_d_full * SC / EW)[None, :]).astype(np.float32)
    w6r = w6s.reshape(4, 128, NW, WT)
    w6blob = np.zeros((128, NW * 4 * WT), np.float32)
    for i in range(NW):
        i4, w = divmod(i, 4)
        for kt in range(4):
            o = (i4 * 4 + w) * 4 * WT + kt * WT
            w6blob[:, o:o + WT] = w6r[kt, :, i, :]
    w6blob = w6blob.astype(F8_NP)

    # wxa shared part: W1 (x64) + folded b1 row
    wxas = np.zeros((128, WXA), np.float32)
    W1 = np.asarray(inputs["W1"], np.float32)
    for kt in range(5):
        nr = min(128, D_IN - 128 * kt)
        wxas[:nr, OW1 + kt * 512:OW1 + (kt + 1) * 512] = \
            W1[128 * kt:128 * kt + nr, :] * EW
    wxas[88, OW1 + 4 * 512:OW1 + 5 * 512] = \
        np.asarray(inputs["b1"], np.float32) * EW
    wxas = wxas.astype(F8_NP)

    # wxb shared part: W2 (x64), W3 (x8), W4 (x64, b4 folded), W5 (x64)
    wxbs = np.zeros((128, WXB), np.float32)
    W2 = np.asarray(inputs["W2"], np.float32) * EW
    for kt in range(4):
        wxbs[:, OW2 + kt * 256:OW2 + (kt + 1) * 256] = W2[kt * 128:(kt + 1) * 128]
    W3 = np.asarray(inputs["W3"], np.float32) * 8.0
    for kt in range(2):
        wxbs[:, OW3 + kt * 128:OW3 + (kt + 1) * 128] = W3[kt * 128:(kt + 1) * 128]
    W4 = np.asarray(inputs["W4"], np.float32) * EW
    wxbs[:, OW4:OW4 + 256] = W4[:128]
    wxbs[0:16, OW4 + 256:OW4 + 512] = W4[128:144]
    wxbs[16, OW4 + 256:OW4 + 512] = np.asarray(inputs["b4"], np.float32) * EW
    W5 = np.asarray(inputs["W5"], np.float32) * EW
    for kt in range(2):
        wxbs[:, OW5 + kt * 512:OW5 + (kt + 1) * 512] = W5[kt * 128:(kt + 1) * 128]
    wxbs = wxbs.astype(F8_NP)

    # bias blob [128, 26] f32: col 0 zero, 17-18 = 64*b2, 19 = b3, 22-25 = 64*b5
    biasb = np.zeros((128, 26), np.float32)
    biasb[:, 17:19] = np.asarray(inputs["b2"], np.float32).reshape(2, 128).T * EW
    biasb[:, 19] = np.asarray(inputs["b3"], np.float32)
    biasb[:, 22:26] = np.asarray(inputs["b5"], np.float32).reshape(4, 128).T * EW

    in_maps = []
    for c in range(NCORES):
        sl = slice(c * BC, (c + 1) * BC)
        xc = x_full[sl]  # [BC, 600]
        xw = np.zeros((128, NI4 * 512), np.float32)
        for i in range(NW):
            i4, wpos = divmod(i, 4)
            p0 = 32 * wpos
            ncols = min(30, D_IN - 24 * i)
            xw[p0:p0 + ncols, i4 * 512:i4 * 512 + BC] = xc[:, 24 * i:24 * i + ncols].T
            xw[p0 + 30, i4 * 512:i4 * 512 + BC] = 1.0
        # xbg blob: per-group blocks [x_g (512) | G_g (480)] for split loads
        xbg = np.zeros((128, NI4 * 992), BF16_NP)
        for g in range(NI4):
            xbg[:, 992 * g:992 * g + 512] = xw[:, 512 * g:512 * (g + 1)].astype(BF16_NP)
            xbg[:, 992 * g + 512:992 * (g + 1)] = gmat[:, WT * g:WT * (g + 1)]
        # wxa: compact x for L1 (ktile k = x cols 128k..128k+127) + W1
        wxa = wxas.copy()
        x8c = np.zeros((128, 5 * 512), np.float32)
        for kt in range(5):
            nr = min(128, D_IN - 128 * kt)
            x8c[:nr, kt * 512:kt * 512 + BC] = xc[:, 128 * kt:128 * kt + nr].T
        x8c[88, 4 * 512:5 * 512] = 1.0  # b1 row
        wxa[:, XO:XO + 2560] = x8c.astype(F8_NP)
        # wxb: shared weights + per-core l4emb ktile
        wxb = wxbs.copy()
        l4emb = np.zeros((128, BC), np.float32)
        l4emb[0:LBL_DIM] = emb[labels[sl]].T
        l4emb[16] = 1.0  # b4 row
        wxb[:, OL4E:OL4E + BC] = l4emb.astype(F8_NP)
        m = {"biasb": biasb, "w6p": w6blob, "wxa": wxa, "wxb": wxb,
             "xbg": xbg}
        in_maps.append(m)
    return in_maps


_NC_CACHE = None


def kernel(**inputs) -> np.ndarray:
    global _NC_CACHE
    if _NC_CACHE is None:
        _NC_CACHE = _build_nc()
    nc = _NC_CACHE
    in_maps = _host_prep(inputs)
    res = bass_utils.run_bass_kernel_spmd(nc, in_maps, core_ids=list(range(NCORES)))
    out = np.concatenate([res.results[c]["y"] for c in range(NCORES)], axis=0)
    return out.astype(np.float32).reshape(B, HIGH_T, FEAT)
